# revision 8
# baseline (speedup 1.0000x reference)
"""PhysicsGAT (3-layer GATConv + BN/ELU + GRIN max-pool) on 8 Trainium2 cores.

Edges sorted by dst, dst-node ranges sharded across 8 cores (edge-balanced).
Per 128-dst-node group: dma_gather of [h|asrc|adst] table rows by src (two
half-tables for int16 idx), per-subtile edge-attr matmul, batched logit
pipeline, selector-matrix segment-sum matmuls accumulated in PSUM.  BN stats
via small AllReduce; next-layer node table via per-shard matmul + AllGather.
Pooling emits per-(group,piece) channel maxima; host max-combines to [64,64].
"""
import sys
if '/opt/trn_rl_repo' not in sys.path:
    sys.path.insert(0, '/opt/trn_rl_repo')

import os
import numpy as np

import concourse.bass as bass
import concourse.bacc as bacc
import concourse.mybir as mybir
import concourse.tile as tile
from concourse import library_config
from concourse.bass_utils import run_bass_kernel_spmd

P = 128
NCORES = 8
AluOp = mybir.AluOpType
AF = mybir.ActivationFunctionType
F32 = mybir.dt.float32
I16 = mybir.dt.int16
I32 = mybir.dt.int32

_CACHE = {}
LAST_EXEC_NS = [None]


def _ru(x, m):
    return -(-x // m) * m


# ---------------------------------------------------------------- host prep
def build_plan(x, edge_index, edge_attr, batch, repeat_unit_mask,
               W1, att_s1, att_d1, We1, att_e1, b1, g1, be1,
               W2, att_s2, att_d2, We2, att_e2, b2, g2, be2,
               W3, att_s3, att_d3, We3, att_e3, b3):
    N, IN = x.shape
    E, ED = edge_attr.shape
    HID = W1.shape[1]
    OUT = W3.shape[1]
    H = att_s1.shape[0]
    B = 64
    Etot = E + N

    src = np.concatenate([np.asarray(edge_index[0], np.int64),
                          np.arange(N, dtype=np.int64)])
    dst = np.concatenate([np.asarray(edge_index[1], np.int64),
                          np.arange(N, dtype=np.int64)])
    isloop = np.concatenate([np.zeros(E, bool), np.ones(N, bool)])

    order = np.argsort(dst, kind='stable')
    src_s, dst_s, loop_s, orig_s = src[order], dst[order], isloop[order], order

    deg = np.bincount(dst, minlength=N)
    cum = np.concatenate([[0], np.cumsum(deg)])
    nlo = [0]
    for c in range(1, NCORES):
        nlo.append(int(np.searchsorted(cum, c * Etot / NCORES)))
    nlo.append(N)
    nlo = np.maximum.accumulate(np.array(nlo))
    nsh = np.diff(nlo)
    Nsh = int(_ru(int(nsh.max()), P))
    NG = Nsh // P
    NTBL = NCORES * Nsh
    HALFR = NTBL // 2
    assert HALFR < 32768, f"half-table {HALFR} rows exceeds int16"

    catof = np.empty(N, np.int64)
    for c in range(NCORES):
        catof[nlo[c]:nlo[c + 1]] = c * Nsh + np.arange(nsh[c])
    cat_src_s = catof[src_s]
    isA_s = cat_src_s < HALFR

    nA = np.zeros((NCORES, NG), np.int64)
    nB = np.zeros((NCORES, NG), np.int64)
    runs = {}
    for c in range(NCORES):
        for g in range(NG):
            glo = nlo[c] + P * g
            ghi = min(glo + P, nlo[c + 1])
            if glo >= ghi:
                runs[(c, g)] = (0, 0)
                continue
            e0, e1 = int(cum[glo]), int(cum[ghi])
            runs[(c, g)] = (e0, e1)
            a = int(isA_s[e0:e1].sum())
            nA[c, g] = a
            nB[c, g] = (e1 - e0) - a

    NIDXA = np.zeros(NG, np.int64)
    NIDXB = np.zeros(NG, np.int64)
    for g in range(NG):
        ma, mb = int(nA[:, g].max()), int(nB[:, g].max())
        NIDXA[g] = _ru(ma, P) if ma > 0 else 0
        NIDXB[g] = _ru(mb, P) if mb > 0 else 0
        if NIDXA[g] + NIDXB[g] == 0:
            NIDXA[g] = P
    SUBT = (NIDXA + NIDXB) // P
    KA = NIDXA // P
    st_off = np.concatenate([[0], np.cumsum(SUBT)])[:-1]
    ST = int(SUBT.sum())
    E_pad = ST * P
    sa16 = np.concatenate([[0], np.cumsum((NIDXA + NIDXB) // 16)])[:-1]

    gidx16 = np.zeros((NCORES, 16, E_pad // 16), np.int16)
    dl_dev = np.zeros((NCORES, P, ST), np.float32)
    ea_cat = np.zeros((NCORES, 8, E_pad), np.float32)
    valid = np.zeros((NCORES, P, NG), np.float32)
    pm = np.zeros((NCORES, P, NG, 2), np.float32)
    pool_map = np.full((NCORES, NG, 2), -1, np.int64)

    mid = (np.asarray(repeat_unit_mask) == 1)
    batch = np.asarray(batch)
    edge_attr = np.asarray(edge_attr, np.float32)

    for c in range(NCORES):
        for g in range(NG):
            e0, e1 = runs[(c, g)]
            na_g, nb_g = int(NIDXA[g]), int(NIDXB[g])
            tot = na_g + nb_g
            ecol = int(st_off[g]) * P
            vals = np.zeros(tot, np.int64)
            dlv = np.zeros(tot, np.int64)
            eav = np.zeros((8, tot), np.float32)
            eav[6, :] = 1.0  # pad by default
            if e1 > e0:
                seg_src = cat_src_s[e0:e1]
                seg_dst = dst_s[e0:e1]
                seg_loop = loop_s[e0:e1]
                seg_orig = orig_s[e0:e1]
                amask = isA_s[e0:e1]
                for half, hmask, base, off in ((0, amask, 0, 0),
                                               (1, ~amask, HALFR, na_g)):
                    if (half == 0 and na_g == 0) or (half == 1 and nb_g == 0):
                        continue
                    sel = np.nonzero(hmask)[0]
                    k = len(sel)
                    if k == 0:
                        continue
                    vals[off:off + k] = seg_src[sel] - base
                    dlv[off:off + k] = seg_dst[sel] - (nlo[c] + P * g)
                    lo = seg_loop[sel]
                    og = seg_orig[sel]
                    eav[6, off:off + k] = 0.0
                    eav[7, off:off + k] = lo.astype(np.float32)
                    real = ~lo
                    if real.any():
                        block = np.zeros((ED, k), np.float32)
                        block[:, real] = edge_attr[og[real]].T
                        eav[0:ED, off:off + k] = block
            gidx16[c, :, int(sa16[g]):int(sa16[g]) + tot // 16] = \
                vals.reshape(tot // 16, 16).T
            dl_dev[c, :, int(st_off[g]):int(st_off[g]) + int(SUBT[g])] = \
                dlv.reshape(int(SUBT[g]), P).T
            ea_cat[c, :, ecol:ecol + tot] = eav
            glo = nlo[c] + P * g
            ghi = min(glo + P, nlo[c + 1])
            nreal = max(0, ghi - glo)
            if nreal > 0:
                valid[c, :nreal, g] = 1.0
                bs = batch[glo:ghi]
                gset = np.unique(bs)
                assert len(gset) <= 2, "group spans >2 graphs"
                for r, gid in enumerate(gset):
                    rows = np.nonzero(bs == gid)[0]
                    rows = rows[mid[glo:ghi][rows]]
                    pool_map[c, g, r] = gid
                    pm[c, rows, g, r] = 1.0

    gidx_dev = np.tile(gidx16, (1, 8, 1)).astype(np.int16)
    eap = ea_cat.reshape(NCORES, 8, ST, P).transpose(0, 3, 2, 1) \
                .reshape(NCORES, P, ST * 8).copy()
    pmb = ((pm - 1.0) * 1e30).astype(np.float32)

    def fold(W, att):
        hh, cc = att.shape
        return np.einsum('fhc,hc->fh', np.asarray(W, np.float32)
                         .reshape(W.shape[0], hh, cc),
                         np.asarray(att, np.float32))

    def wcat(W, atts, attd, width):
        f, wout = np.asarray(W).shape
        out = np.zeros((f, width), np.float32)
        out[:, :wout] = W
        hh = np.asarray(atts).shape[0]
        out[:, wout:wout + hh] = fold(W, atts)
        out[:, wout + hh:wout + 2 * hh] = fold(W, attd)
        return out

    wcat1 = wcat(W1, att_s1, att_d1, 192)
    wcat2 = wcat(W2, att_s2, att_d2, 192)
    wcat3 = wcat(W3, att_s3, att_d3, 128)

    def weatt(We, att):
        out = np.zeros((8, 4), np.float32)
        f = fold(We, att)
        out[:ED, :f.shape[1]] = f
        out[6, :] = -1e9          # pad-edge kill
        return out

    wef1, wef2, wef3 = weatt(We1, att_e1), weatt(We2, att_e2), weatt(We3, att_e3)

    xT_cat = np.zeros((IN, NTBL), np.float32)
    xT_cat[:, catof] = np.asarray(x, np.float32).T
    xT_own = np.stack([xT_cat[:, c * Nsh:(c + 1) * Nsh] for c in range(NCORES)])

    row1 = lambda v, w: np.asarray(v, np.float32).reshape(1, w)
    in_maps = []
    for c in range(NCORES):
        in_maps.append({
            'xT_cat': xT_cat, 'xT_own': xT_own[c],
            'gidx': gidx_dev[c], 'dl': dl_dev[c], 'eac': ea_cat[c],
            'eap': eap[c], 'valid': valid[c],
            'pm': pm[c].reshape(P, NG * 2), 'pmb': pmb[c].reshape(P, NG * 2),
            'wcat1': wcat1, 'wcat2': wcat2, 'wcat3': wcat3,
            'wef1': wef1, 'wef2': wef2, 'wef3': wef3,
            'b1': row1(b1, HID), 'b2': row1(b2, HID), 'b3': row1(b3, OUT),
            'g1': row1(g1, HID), 'be1': row1(be1, HID),
            'g2': row1(g2, HID), 'be2': row1(be2, HID),
        })

    return dict(N=N, E=E, IN=IN, HID=HID, OUT=OUT, H=H, ED=ED, B=B,
                Nsh=Nsh, NG=NG, NTBL=NTBL, HALFR=HALFR, E_pad=E_pad, ST=ST,
                NIDXA=NIDXA, NIDXB=NIDXB, SUBT=SUBT, KA=KA, sa16=sa16,
                st_off=st_off, in_maps=in_maps, pool_map=pool_map)


# ---------------------------------------------------------------- program
def build_program(plan):
    N, IN, HID, OUT, H = plan['N'], plan['IN'], plan['HID'], plan['OUT'], plan['H']
    ED = plan['ED']
    Nsh, NG, NTBL, HALFR = plan['Nsh'], plan['NG'], plan['NTBL'], plan['HALFR']
    E_pad, ST = plan['E_pad'], plan['ST']
    NIDXA, NIDXB, SUBT, KA = plan['NIDXA'], plan['NIDXB'], plan['SUBT'], plan['KA']
    sa16, st_off = plan['sa16'], plan['st_off']
    SUBTM = int(SUBT.max())
    C = HID // H

    nc = bacc.Bacc("TRN2", target_bir_lowering=False, debug=False,
                   num_devices=NCORES)
    din = lambda nm, sh, dt=F32: nc.dram_tensor(nm, sh, dt, kind="ExternalInput")
    xT_cat = din('xT_cat', [IN, NTBL])
    xT_own = din('xT_own', [IN, Nsh])
    gidx = din('gidx', [P, E_pad // 16], I16)
    dl = din('dl', [P, ST])
    eac = din('eac', [8, E_pad])
    eap = din('eap', [P, ST * 8])
    valid = din('valid', [P, NG])
    pm_in = din('pm', [P, NG * 2])
    pmb_in = din('pmb', [P, NG * 2])
    wcat1 = din('wcat1', [IN, 192])
    wcat2 = din('wcat2', [HID, 192])
    wcat3 = din('wcat3', [HID, 128])
    wef_in = [din('wef1', [8, 4]), din('wef2', [8, 4]), din('wef3', [8, 4])]
    b_in = [din('b1', [1, HID]), din('b2', [1, HID]), din('b3', [1, OUT])]
    gg_in = [din('g1', [1, HID]), din('g2', [1, HID])]
    be_in = [din('be1', [1, HID]), din('be2', [1, HID])]
    pool_out = nc.dram_tensor('pool', [64, NG * 2], F32, kind="ExternalOutput")

    rg = [list(range(NCORES))]

    with tile.TileContext(nc) as tc:
        with tc.tile_pool(name="cst", bufs=1) as cst, \
             tc.tile_pool(name="wk", bufs=2) as wk, \
             tc.tile_pool(name="wk1", bufs=1) as wk1, \
             tc.tile_pool(name="one", bufs=1) as one, \
             tc.tile_pool(name="ps", bufs=1, space="PSUM") as ps, \
             tc.tile_pool(name="ps2", bufs=2, space="PSUM") as ps2, \
             tc.tile_pool(name="dram", bufs=1, space="DRAM") as dr:
            nc.gpsimd.load_library(library_config.mlp)

            T1 = dr.tile([NTBL, 192], F32)
            Tsh = [dr.tile([Nsh, 192], F32, name="Tsh0"),
                   dr.tile([Nsh, 192], F32, name="Tsh1"),
                   dr.tile([Nsh, 128], F32, name="Tsh2")]
            T2 = dr.tile([NTBL, 192], F32)
            T3 = dr.tile([NTBL, 128], F32)
            Ttab = [T1, T2, T3]
            stats_i = dr.tile([1, 256], F32)
            stats_o = dr.tile([1, 256], F32)
            eam_i = dr.tile([1, 8], F32)
            eam_o = dr.tile([1, 8], F32)
            eamT_d = dr.tile([8, 1], F32)
            alp_d = dr.tile([1, 4], F32)
            wef_scr = [dr.tile([8, 4], F32, name="wefs0"),
                       dr.tile([8, 4], F32, name="wefs1"),
                       dr.tile([8, 4], F32, name="wefs2")]
            ab_d = dr.tile([1, 256], F32)

            gidx_t = cst.tile([P, E_pad // 16], I16)
            nc.sync.dma_start(gidx_t[:], gidx[:])
            dl_t = cst.tile([P, ST], F32)
            nc.sync.dma_start(dl_t[:], dl[:])
            valid_t = cst.tile([P, NG], F32)
            nc.sync.dma_start(valid_t[:], valid[:])
            pm_t = cst.tile([P, NG * 2], F32)
            nc.sync.dma_start(pm_t[:], pm_in[:])
            pmb_t = cst.tile([P, NG * 2], F32)
            nc.sync.dma_start(pmb_t[:], pmb_in[:])
            wcat1_t = cst.tile([IN, 192], F32)
            nc.sync.dma_start(wcat1_t[:], wcat1[:])
            wcat2_t = cst.tile([HID, 192], F32)
            nc.sync.dma_start(wcat2_t[:], wcat2[:])
            wcat3_t = cst.tile([HID, 128], F32)
            nc.sync.dma_start(wcat3_t[:], wcat3[:])
            bias_r = []
            for l, w in ((0, HID), (1, HID), (2, OUT)):
                t = cst.tile([P, w], F32, tag=f"biasr{l}")
                nc.sync.dma_start(t[:], b_in[l][:].to_broadcast((P, w)))
                bias_r.append(t)
            gbe_t = []
            for l in range(2):
                gt_ = cst.tile([1, HID], F32, tag=f"g{l}")
                nc.sync.dma_start(gt_[:], gg_in[l][:])
                bt_ = cst.tile([1, HID], F32, tag=f"be{l}")
                nc.sync.dma_start(bt_[:], be_in[l][:])
                gbe_t.append((gt_, bt_))
            iota_i = cst.tile([P, P], I32)
            nc.gpsimd.iota(iota_i[:], pattern=[[1, P]], channel_multiplier=0)
            iota_f = cst.tile([P, P], F32)
            nc.vector.tensor_copy(iota_f[:], iota_i[:])
            ident = cst.tile([P, P], F32)
            from concourse.masks import make_identity
            make_identity(nc, ident[:])
            ones_t = cst.tile([P, 1], F32)
            nc.vector.memset(ones_t[:], 1.0)
            hraw = cst.tile([P, NG, HID], F32)
            nc.vector.memset(hraw[:], 0.0)

            # ---------- ea mean over the E original edges ----------
            eam_red = one.tile([1, 8], F32)
            nc.vector.memset(eam_red[:], 0.0)
            CH = 4096
            for j0 in range(0, ST * 8, CH):
                w = min(CH, ST * 8 - j0)
                ch_t = wk.tile([P, SUBTM * 192], F32, tag="gt")
                nc.sync.dma_start(ch_t[:, :w], eap[:, j0:j0 + w])
                for k0 in range(0, w, 512):
                    kw = min(512, w - k0)
                    mps = ps.tile([1, 512], F32, tag="sc_ps", space="PSUM")
                    nc.tensor.matmul(mps[:, :kw], lhsT=ones_t[:],
                                     rhs=ch_t[:, k0:k0 + kw], start=True, stop=True)
                    part = wk.tile([1, 8], F32, tag="eam_p")
                    nc.vector.reduce_sum(
                        part[:], mps[:, :kw].rearrange("a (blk d) -> a d blk", d=8),
                        axis=mybir.AxisListType.X)
                    nc.vector.tensor_tensor(out=eam_red[:], in0=eam_red[:],
                                            in1=part[:], op=AluOp.add)
            nc.vector.tensor_scalar(out=eam_red[:], in0=eam_red[:],
                                    scalar1=1.0 / plan['E'], scalar2=None,
                                    op0=AluOp.mult)
            nc.sync.dma_start(eam_i[:], eam_red[:])
            nc.gpsimd.collective_compute("AllReduce", AluOp.add,
                                         replica_groups=rg,
                                         ins=[eam_i.opt()], outs=[eam_o.opt()])
            nc.sync.dma_start(eamT_d[:], eam_o[:].rearrange("a d -> d a"))
            eamT_t = one.tile([8, 1], F32)
            nc.sync.dma_start(eamT_t[:], eamT_d[:])

            # ---------- layer-1 tables ----------
            def phase_a1(xT_ap, dst_dram, nrows):
                nblk = nrows // P
                for j4 in range(0, nblk, 4):
                    jn = min(4, nblk - j4)
                    tb = wk.tile([P, 4, 192], F32, tag="phA4")
                    for j in range(j4, j4 + jn):
                        aps = ps.tile([P, 192], F32, tag="trpaps", space="PSUM")
                        xs = wk.tile([IN, P], F32, tag="xs")
                        nc.sync.dma_start(xs[:], xT_ap[:, j * P:(j + 1) * P])
                        nc.tensor.matmul(aps[:], lhsT=xs[:], rhs=wcat1_t[:],
                                         start=True, stop=True)
                        nc.vector.tensor_copy(tb[:, j - j4, :], aps[:])
                    nc.sync.dma_start(
                        dst_dram[j4 * P:(j4 + jn) * P, :]
                        .rearrange("(j p) w -> p j w", p=P),
                        tb[:, :jn, :])

            phase_a1(xT_cat[:], T1, NTBL)
            phase_a1(xT_own[:], Tsh[0], Nsh)

            # ---------- layers ----------
            for l in range(3):
                ELEM = 192 if l < 2 else 128
                HW = HID if l < 2 else OUT
                HL = H if l < 2 else 1
                CL = C if l < 2 else OUT
                asrc_c = HW
                adst_c = HW + HL

                if l > 0:
                    nc.gpsimd.collective_compute(
                        "AllGather", AluOp.bypass, replica_groups=rg,
                        ins=[Tsh[l].opt()], outs=[Ttab[l].opt()])

                wef0_t = one.tile([8, 4], F32, tag=f"wef0_{l}")
                nc.sync.dma_start(wef0_t[:], wef_in[l][:])
                alp_ps = ps.tile([1, 512], F32, tag="sc_ps", space="PSUM")
                nc.tensor.matmul(alp_ps[:, 0:4], lhsT=eamT_t[0:ED, :],
                                 rhs=wef0_t[0:ED, :], start=True, stop=True)
                alp_sb = wk.tile([1, 8], F32, tag="eam_p")
                nc.vector.tensor_copy(alp_sb[:, 0:4], alp_ps[:, 0:4])
                nc.sync.dma_start(wef_scr[l][:], wef_in[l][:])
                nc.sync.dma_start(wef_scr[l][7:8, :], alp_sb[:, 0:4])
                wef_t = one.tile([8, 4], F32, tag=f"wef{l}")
                nc.sync.dma_start(wef_t[:], wef_scr[l][:])

                adsh = one.tile([P, NG, 4], F32, tag=f"adsh{l}")
                nc.sync.dma_start(
                    adsh[:, :, 0:HL],
                    Tsh[l][:].rearrange("(g p) w -> p g w", p=P)
                    [:, :, adst_c:adst_c + HL])

                if l < 2:
                    st_ps = ps.tile([1, 128], F32, tag="stats_h", space="PSUM")
                    st_ps2 = ps.tile([1, 128], F32, tag="stats_sq", space="PSUM")

                for g in range(NG):
                    subt = int(SUBT[g])
                    ka = int(KA[g])
                    na, nb = int(NIDXA[g]), int(NIDXB[g])
                    gt = wk.tile([P, SUBTM, ELEM], F32, tag="gt")
                    if na:
                        nc.gpsimd.dma_gather(
                            out_ap=gt[:, 0:ka, :], in_ap=Ttab[l][0:HALFR, :],
                            idxs_ap=gidx_t[:, int(sa16[g]):int(sa16[g]) + na // 16],
                            num_idxs=na, num_idxs_reg=na, elem_size=ELEM,
                            single_packet=False)
                    if nb:
                        nc.gpsimd.dma_gather(
                            out_ap=gt[:, ka:subt, :], in_ap=Ttab[l][HALFR:NTBL, :],
                            idxs_ap=gidx_t[:, int(sa16[g]) + na // 16:
                                           int(sa16[g]) + (na + nb) // 16],
                            num_idxs=nb, num_idxs_reg=nb, elem_size=ELEM,
                            single_packet=False)
                    ecol = int(st_off[g]) * P
                    eag = wk1.tile([8, SUBTM * P], F32, tag="eag")
                    nc.sync.dma_start(eag[:, :subt * P], eac[:, ecol:ecol + subt * P])
                    qps = ps.tile([P, SUBTM, 4], F32, tag="qps", space="PSUM")
                    for k in range(subt):
                        nc.tensor.matmul(qps[:, k, :],
                                         lhsT=eag[:, k * P:(k + 1) * P],
                                         rhs=wef_t[:], start=True, stop=True)
                    s01 = wk1.tile([P, SUBTM, P], F32, tag="s01")
                    nc.vector.tensor_tensor(
                        out=s01[:, :subt, :],
                        in0=dl_t[:, int(st_off[g]):int(st_off[g]) + subt]
                        [:, :, None].to_broadcast((P, subt, P)),
                        in1=iota_f[:, None, :].to_broadcast((P, subt, P)),
                        op=AluOp.is_equal)
                    adps = ps.tile([P, SUBTM, 4], F32, tag="adps", space="PSUM")
                    for k in range(subt):
                        trs = ps.tile([P, 192], F32, tag="trpaps",
                                      space="PSUM", name="trs")[:, 0:128]
                        nc.tensor.transpose(trs[:], s01[:, k, :], ident[:])
                        s01T = wk.tile([P, P], F32, tag="s01T")
                        nc.scalar.copy(s01T[:], trs[:])
                        nc.tensor.matmul(adps[:, k, 0:HL], lhsT=s01T[:],
                                         rhs=adsh[:, g, 0:HL],
                                         start=True, stop=True)
                    s1 = wk.tile([P, SUBTM, 4], F32, tag="s1")
                    s2 = wk.tile([P, SUBTM, 4], F32, tag="s2")
                    nc.vector.tensor_tensor(
                        out=s1[:, :subt, 0:HL], in0=qps[:, :subt, 0:HL],
                        in1=gt[:, :subt, asrc_c:asrc_c + HL], op=AluOp.add)
                    nc.vector.tensor_tensor(
                        out=s1[:, :subt, 0:HL], in0=s1[:, :subt, 0:HL],
                        in1=adps[:, :subt, 0:HL], op=AluOp.add)
                    nc.vector.tensor_scalar(out=s2[:, :subt, 0:HL],
                                            in0=s1[:, :subt, 0:HL],
                                            scalar1=0.2, scalar2=None,
                                            op0=AluOp.mult)
                    nc.vector.tensor_tensor(out=s1[:, :subt, 0:HL],
                                            in0=s1[:, :subt, 0:HL],
                                            in1=s2[:, :subt, 0:HL], op=AluOp.max)
                    msgw = wk1.tile([P, SUBTM, HW + HL], F32, tag="msgw")
                    nc.scalar.activation(msgw[:, :subt, HW:HW + HL],
                                         s1[:, :subt, 0:HL], AF.Exp)
                    nc.vector.tensor_tensor(
                        out=msgw[:, :subt, 0:HW]
                        .rearrange("p k (h c) -> p k h c", h=HL),
                        in0=gt[:, :subt, 0:HW]
                        .rearrange("p k (h c) -> p k h c", h=HL),
                        in1=msgw[:, :subt, HW:HW + HL][:, :, :, None]
                        .to_broadcast((P, subt, HL, CL)),
                        op=AluOp.mult)
                    acc = ps2.tile([P, HW + HL], F32, tag="acc", space="PSUM")
                    for k in range(subt):
                        nc.tensor.matmul(acc[:], lhsT=s01[:, k, :],
                                         rhs=msgw[:, k, :],
                                         start=(k == 0), stop=(k == subt - 1))
                    rec = wk.tile([P, 4], F32, tag="rec")
                    nc.vector.tensor_scalar(out=rec[:, 0:HL],
                                            in0=acc[:, HW:HW + HL],
                                            scalar1=1e-16, scalar2=None,
                                            op0=AluOp.add)
                    nc.vector.reciprocal(rec[:, 0:HL], rec[:, 0:HL])
                    nc.vector.tensor_tensor(
                        out=hraw[:, g, 0:HW].rearrange("p (h c) -> p h c", h=HL),
                        in0=acc[:, 0:HW].rearrange("p (h c) -> p h c", h=HL),
                        in1=rec[:, 0:HL][:, :, None].to_broadcast((P, HL, CL)),
                        op=AluOp.mult)
                    nc.vector.tensor_tensor(out=hraw[:, g, 0:HW],
                                            in0=hraw[:, g, 0:HW],
                                            in1=bias_r[l][:], op=AluOp.add)
                    if l < 2:
                        sq = wk.tile([P, HID], F32, tag="sq")
                        nc.vector.tensor_tensor(out=sq[:], in0=hraw[:, g, :],
                                                in1=hraw[:, g, :], op=AluOp.mult)
                        nc.tensor.matmul(st_ps[:],
                                         lhsT=valid_t[:, g:g + 1],
                                         rhs=hraw[:, g, :],
                                         start=(g == 0), stop=(g == NG - 1))
                        nc.tensor.matmul(st_ps2[:],
                                         lhsT=valid_t[:, g:g + 1], rhs=sq[:],
                                         start=(g == 0), stop=(g == NG - 1))

                if l < 2:
                    st_sb = wk.tile([1, 256], F32, tag="st_sb")
                    nc.vector.tensor_copy(st_sb[:, 0:128], st_ps[:])
                    nc.vector.tensor_copy(st_sb[:, 128:256], st_ps2[:])
                    nc.sync.dma_start(stats_i[:], st_sb[:])
                    nc.gpsimd.collective_compute("AllReduce", AluOp.add,
                                                 replica_groups=rg,
                                                 ins=[stats_i.opt()],
                                                 outs=[stats_o.opt()])
                    st2 = wk.tile([1, 256], F32, tag="st2")
                    nc.sync.dma_start(st2[:], stats_o[:])
                    mu = wk.tile([1, HID], F32, tag="mu")
                    var = wk.tile([1, HID], F32, tag="var")
                    nc.vector.tensor_scalar(out=mu[:], in0=st2[:, 0:128],
                                            scalar1=1.0 / N, scalar2=None,
                                            op0=AluOp.mult)
                    nc.vector.tensor_scalar(out=var[:], in0=st2[:, 128:256],
                                            scalar1=1.0 / N, scalar2=None,
                                            op0=AluOp.mult)
                    musq = wk.tile([1, HID], F32, tag="musq")
                    nc.vector.tensor_tensor(out=musq[:], in0=mu[:], in1=mu[:],
                                            op=AluOp.mult)
                    nc.vector.tensor_tensor(out=var[:], in0=var[:], in1=musq[:],
                                            op=AluOp.subtract)
                    nc.vector.tensor_scalar(out=var[:], in0=var[:],
                                            scalar1=1e-5, scalar2=None,
                                            op0=AluOp.add)
                    nc.scalar.activation(var[:], var[:], AF.Sqrt)
                    nc.vector.reciprocal(var[:], var[:])
                    nc.vector.tensor_tensor(out=var[:], in0=var[:],
                                            in1=gbe_t[l][0][:], op=AluOp.mult)
                    nc.vector.tensor_tensor(out=musq[:], in0=mu[:], in1=var[:],
                                            op=AluOp.mult)
                    nc.vector.tensor_tensor(out=musq[:], in0=gbe_t[l][1][:],
                                            in1=musq[:], op=AluOp.subtract)
                    ab_sb = wk.tile([1, 256], F32, tag="ab_sb")
                    nc.vector.tensor_copy(ab_sb[:, 0:128], var[:])
                    nc.vector.tensor_copy(ab_sb[:, 128:256], musq[:])
                    nc.sync.dma_start(ab_d[:], ab_sb[:])
                    a_rep = wk.tile([P, HID], F32, tag="a_rep")
                    nc.sync.dma_start(a_rep[:],
                                      ab_d[:, 0:128].to_broadcast((P, HID)))
                    bb_rep = wk.tile([P, HID], F32, tag="bb_rep")
                    nc.sync.dma_start(bb_rep[:],
                                      ab_d[:, 128:256].to_broadcast((P, HID)))
                    nc.vector.tensor_tensor(
                        out=hraw[:], in0=hraw[:],
                        in1=a_rep[:, None, :].to_broadcast((P, NG, HID)),
                        op=AluOp.mult)
                    nc.vector.tensor_tensor(
                        out=hraw[:], in0=hraw[:],
                        in1=bb_rep[:, None, :].to_broadcast((P, NG, HID)),
                        op=AluOp.add)
                    wout2 = 192 if l == 0 else 128
                    wnext = wcat2_t if l == 0 else wcat3_t
                    for g in range(NG):
                        t1 = wk.tile([P, HID], F32, tag="elu1")
                        nc.vector.tensor_scalar(out=t1[:], in0=hraw[:, g, :],
                                                scalar1=0.0, scalar2=None,
                                                op0=AluOp.min)
                        nc.scalar.activation(t1[:], t1[:], AF.Exp)
                        nc.vector.tensor_scalar(out=t1[:], in0=t1[:],
                                                scalar1=-1.0, scalar2=None,
                                                op0=AluOp.add)
                        xg = wk.tile([P, HID], F32, tag="xg")
                        nc.vector.tensor_tensor(out=xg[:], in0=hraw[:, g, :],
                                                in1=t1[:], op=AluOp.max)
                        trp = ps.tile([P, 192], F32, tag="trpaps", space="PSUM", name="trp")[:, 0:128]
                        nc.tensor.transpose(trp[:], xg[:], ident[:])
                        xTs = wk.tile([P, P], F32, tag="xTs")
                        nc.vector.tensor_copy(xTs[:], trp[:])
                        aps = ps.tile([P, 192], F32, tag="trpaps", space="PSUM")
                        nc.tensor.matmul(aps[:, 0:wout2], lhsT=xTs[:],
                                         rhs=wnext[:], start=True, stop=True)
                        tb = wk.tile([P, 192], F32, tag="phA")
                        nc.vector.tensor_copy(tb[:, 0:wout2], aps[:, 0:wout2])
                        nc.sync.dma_start(Tsh[l + 1][g * P:(g + 1) * P, :],
                                          tb[:, 0:wout2])
                else:
                    pool_sb = one.tile([64, NG * 2], F32, tag="pool")
                    for g in range(NG):
                        t1 = wk.tile([P, HID], F32, tag="elu1")
                        nc.vector.tensor_scalar(out=t1[:, 0:OUT],
                                                in0=hraw[:, g, 0:OUT],
                                                scalar1=0.0, scalar2=None,
                                                op0=AluOp.min)
                        nc.scalar.activation(t1[:, 0:OUT], t1[:, 0:OUT], AF.Exp)
                        nc.vector.tensor_scalar(out=t1[:, 0:OUT],
                                                in0=t1[:, 0:OUT],
                                                scalar1=-1.0, scalar2=None,
                                                op0=AluOp.add)
                        xg = wk.tile([P, HID], F32, tag="xg")
                        nc.vector.tensor_tensor(out=xg[:, 0:OUT],
                                                in0=hraw[:, g, 0:OUT],
                                                in1=t1[:, 0:OUT], op=AluOp.max)
                        for r in range(2):
                            vv = wk.tile([P, OUT], F32, tag="vv")
                            nc.vector.tensor_tensor(
                                out=vv[:], in0=xg[:, 0:OUT],
                                in1=pm_t[:, 2 * g + r:2 * g + r + 1]
                                .to_broadcast((P, OUT)), op=AluOp.mult)
                            nc.vector.tensor_tensor(
                                out=vv[:], in0=vv[:],
                                in1=pmb_t[:, 2 * g + r:2 * g + r + 1]
                                .to_broadcast((P, OUT)), op=AluOp.add)
                            trp = ps.tile([P, 192], F32, tag="trpaps", space="PSUM", name="trp")[:, 0:128]
                            nc.tensor.transpose(trp[0:OUT, :], vv[:], ident[:])
                            nc.vector.reduce_max(
                                pool_sb[:, 2 * g + r:2 * g + r + 1],
                                trp[0:64, :], axis=mybir.AxisListType.X)
                    nc.sync.dma_start(pool_out[:], pool_sb[:])
    nc.compile()
    return nc


# ---------------------------------------------------------------- entry
def kernel(**inputs):
    args = {k: np.asarray(v) for k, v in inputs.items() if k != 'num_graphs'}
    key = tuple(sorted((k, v.shape) for k, v in args.items()))
    plan = build_plan(**args)
    if key in _CACHE:
        nc = _CACHE[key]
    else:
        nc = build_program(plan)
        _CACHE[key] = nc

    trace = bool(os.environ.get('GNN_TRACE'))
    res = run_bass_kernel_spmd(nc, plan['in_maps'],
                               core_ids=list(range(NCORES)), trace=trace)
    LAST_EXEC_NS[0] = res.exec_time_ns

    NG, B, OUT = plan['NG'], plan['B'], plan['OUT']
    pool_map = plan['pool_map']
    out = np.full((B, OUT), -np.inf, np.float32)
    for c in range(NCORES):
        part = res.results[c]['pool']          # [64, NG*2]
        for g in range(NG):
            for r in range(2):
                j = pool_map[c, g, r]
                if j < 0:
                    continue
                out[j] = np.maximum(out[j], part[:OUT, 2 * g + r])
    out = np.where(out > -1e29, out, 0.0).astype(np.float32)
    return out
